# revision 7
# baseline (speedup 1.0000x reference)
"""Causal attention (LN -> QKV -> 16-head causal attn -> out-proj) on 8 TRN2 cores.

Sharding: core c = (batch b=c//4, head-group g=c%4). Each core runs its batch's
LayerNorm + a 4-head slice of QKV / attention / out-projection. The out-proj
partials (column-split over the inner dim) are summed on the host per batch.

Device layout notes (per core):
  xnT  [128, 8, 2048]  bf16   normalized input, transposed (DIM on partitions)
  QT/KT pair tiles [128, 2048] bf16  (two heads stacked: head-dim on partitions)
  V    [128, 16, 4, 65] bf16  natural keys-on-partitions layout; 65th col = 1.0
                              so PV's lhsT also accumulates softmax denominators
  S^T  computed per key-block kb as psum [128 keys, 1024 q-half]; exp'd on
       ScalarE straight from PSUM; causal staircase handled by skipping fully
       masked regions + one affine_select on the diagonal 128x128 block.
  outT psum [65, 2048]: rows 0..63 = unnormalized head output (d on partitions),
       row 64 = softmax denominators. recip via exp(-log(d)) (same ACT table
       set as the softmax exp -> no table reloads).
"""

import numpy as np

import concourse.bass as bass
import concourse.mybir as mybir
import concourse.tile as tile
from concourse import bacc
from concourse.bass_utils import run_bass_kernel_spmd
from concourse.masks import make_identity

B, N, DIM, HEADS, DIM_HEAD = 2, 2048, 1024, 16, 64
INNER = HEADS * DIM_HEAD
H_LOC = 4                      # heads per core
N_CORES = 8
P = 128
NB = N // P                    # 16 seq blocks
KB = DIM // P                  # 8 dim blocks
QT = 512                       # psum-bank-sized q tile
HALF = 1024                    # q span per S^T psum tile
SCALE = DIM_HEAD ** -0.5
LN_EPS = 1e-5

F32 = mybir.dt.float32
BF16 = mybir.dt.bfloat16
AF = mybir.ActivationFunctionType
ALU = mybir.AluOpType


def build_nc():
    nc = bacc.Bacc(None, target_bir_lowering=False, debug=False)

    x_d = nc.dram_tensor("x", [N, DIM], F32, kind="ExternalInput")
    wq_d = nc.dram_tensor("wq", [DIM, H_LOC * DIM_HEAD], F32, kind="ExternalInput")
    wk_d = nc.dram_tensor("wk", [DIM, H_LOC * DIM_HEAD], F32, kind="ExternalInput")
    wv_d = nc.dram_tensor("wv", [DIM, H_LOC * DIM_HEAD], F32, kind="ExternalInput")
    wo_d = nc.dram_tensor("wo", [H_LOC * DIM_HEAD, DIM], F32, kind="ExternalInput")
    bq_d = nc.dram_tensor("bq", [P, 2], F32, kind="ExternalInput")
    bk_d = nc.dram_tensor("bk", [P, 2], F32, kind="ExternalInput")
    bv_d = nc.dram_tensor("bv", [1, H_LOC * DIM_HEAD], F32, kind="ExternalInput")
    out_d = nc.dram_tensor("out", [N, DIM], F32, kind="ExternalOutput")

    with tile.TileContext(nc) as tc:
        from contextlib import ExitStack

        ctx = ExitStack()
        with ctx:
            const = ctx.enter_context(tc.tile_pool(name="const", bufs=1))
            persist = ctx.enter_context(tc.tile_pool(name="persist", bufs=1))
            wstage = ctx.enter_context(tc.tile_pool(name="wstage", bufs=2))
            xpool = ctx.enter_context(tc.tile_pool(name="xpool", bufs=3))
            xnpool = ctx.enter_context(tc.tile_pool(name="xnpool", bufs=3))
            stat = ctx.enter_context(tc.tile_pool(name="stat", bufs=4))
            expp = ctx.enter_context(tc.tile_pool(name="expp", bufs=3))
            smalls = ctx.enter_context(tc.tile_pool(name="smalls", bufs=2))
            rbcp = ctx.enter_context(tc.tile_pool(name="rbcp", bufs=2))
            dramp = ctx.enter_context(tc.tile_pool(name="dramp", bufs=2, space="DRAM"))
            stage = ctx.enter_context(tc.tile_pool(name="stage", bufs=3))

            # ---- constants / weights ----
            ident = const.tile([P, P], BF16, tag="ident")
            make_identity(nc, ident)
            eps_t = const.tile([P, 1], F32, tag="eps")
            nc.vector.memset(eps_t, LN_EPS)

            bq_sb = const.tile([P, 2], F32, tag="bq")
            nc.sync.dma_start(bq_sb[:], bq_d[:])
            bk_sb = const.tile([P, 2], F32, tag="bk")
            nc.sync.dma_start(bk_sb[:], bk_d[:])
            bv_sb = const.tile([P, H_LOC, DIM_HEAD], F32, tag="bv")
            nc.sync.dma_start(
                bv_sb[:],
                bv_d[:].rearrange("o (h d) -> o h d", h=H_LOC)
                .to_broadcast((P, H_LOC, DIM_HEAD)),
            )

            def load_w_bf(dram, shape3, tag):
                st = wstage.tile(shape3, F32, tag="wst")
                nc.sync.dma_start(st[:], dram[:].rearrange("(kb p) m -> p kb m", p=P))
                bf = persist.tile(shape3, BF16, tag=tag)
                nc.vector.tensor_copy(bf[:], st[:])
                return bf

            wq_bf = load_w_bf(wq_d, [P, KB, H_LOC * DIM_HEAD], "wq")
            wk_bf = load_w_bf(wk_d, [P, KB, H_LOC * DIM_HEAD], "wk")
            wv_bf = load_w_bf(wv_d, [P, KB, H_LOC * DIM_HEAD], "wv")
            wo_bf = load_w_bf(wo_d, [P, 2, DIM], "wo")

            xnT = persist.tile([P, KB, N], BF16, tag="xnT")
            QTt = [persist.tile([P, N], BF16, tag=f"qt{p_}", name=f"qt{p_}") for p_ in range(2)]
            KTt = [persist.tile([P, N], BF16, tag=f"kt{p_}", name=f"kt{p_}") for p_ in range(2)]
            Vt = persist.tile([P, NB, H_LOC, DIM_HEAD + 1], BF16, tag="v")
            nc.gpsimd.memset(Vt[:], 1.0)  # 65th column stays 1.0 -> denominators
            outT = [persist.tile([P, N], BF16, tag=f"outT{p_}", name=f"outT{p_}") for p_ in range(2)]

            # ---- phase A: LN -> transpose -> QKV -> V ----
            psA_cm = tc.tile_pool(name="psA", bufs=4, space="PSUM")
            psA = psA_cm.__enter__()

            xn_tiles = []
            for sb in range(NB):
                x_t = xpool.tile([P, DIM], F32, tag="x")
                nc.sync.dma_start(x_t[:], x_d[sb * P:(sb + 1) * P, :])

                stats = stat.tile([P, 2, 6], F32, tag="bnst")
                x3 = x_t[:].rearrange("p (a f) -> p a f", a=2)
                for a in range(2):
                    nc.vector.bn_stats(stats[:, a, :], x3[:, a, :])
                mv = stat.tile([P, 2], F32, tag="mv")
                nc.vector.bn_aggr(mv[:], stats[:])
                # rstd = 1/sqrt(var + eps)
                rstd = stat.tile([P, 1], F32, tag="rstd")
                nc.scalar.activation(rstd[:], mv[:, 1:2], AF.Sqrt, bias=eps_t[:])
                nc.vector.reciprocal(rstd[:], rstd[:])
                negmean = stat.tile([P, 1], F32, tag="negmean")
                nc.vector.tensor_scalar_mul(negmean[:], mv[:, 0:1], -1.0)

                xn_bf = xnpool.tile([P, DIM], BF16, tag="xn")
                nc.vector.tensor_scalar(
                    xn_bf[:], x_t[:], negmean[:], rstd[:], ALU.add, ALU.mult
                )
                xn_tiles.append(xn_bf)

                # transpose this seq block: 8 dim-blocks via PE, 2 psum tiles
                for half in range(2):
                    ps = psA.tile([P, 512], F32, tag="ps")
                    for j in range(4):
                        kb = half * 4 + j
                        nc.tensor.matmul(
                            ps[:, j * P:(j + 1) * P],
                            xn_bf[:, kb * P:(kb + 1) * P],
                            ident[:],
                            start=True, stop=True,
                        )
                    dst = xnT[:, half * 4:(half + 1) * 4, sb * P:(sb + 1) * P]
                    src = ps[:].rearrange("p (a f) -> p a f", a=4)
                    if sb % 2 == 0:
                        nc.scalar.copy(dst, src)
                    else:
                        nc.vector.tensor_copy(dst, src)

            # Q^T / K^T per head-pair
            for (wt, bias_sb, dstt) in ((wq_bf, bq_sb, QTt), (wk_bf, bk_sb, KTt)):
                for pr in range(2):
                    for st in range(N // 512):
                        ps = psA.tile([P, 512], F32, tag="ps")
                        for kb in range(KB):
                            nc.tensor.matmul(
                                ps[:],
                                wt[:, kb, pr * P:(pr + 1) * P],
                                xnT[:, kb, st * 512:(st + 1) * 512],
                                start=(kb == 0), stop=(kb == KB - 1),
                            )
                        nc.scalar.activation(
                            dstt[pr][:, st * 512:(st + 1) * 512], ps[:],
                            AF.Identity, bias=bias_sb[:, pr:pr + 1],
                        )

            # V natural layout
            for sb in range(NB):
                ps = psA.tile([P, 512], F32, tag="ps")
                psv = ps[:, :H_LOC * DIM_HEAD]
                for kb in range(KB):
                    nc.tensor.matmul(
                        psv,
                        xnT[:, kb, sb * P:(sb + 1) * P],
                        wv_bf[:, kb, :],
                        start=(kb == 0), stop=(kb == KB - 1),
                    )
                nc.vector.tensor_tensor(
                    Vt[:, sb, :, :DIM_HEAD],
                    psv.rearrange("p (h d) -> p h d", h=H_LOC),
                    bv_sb[:],
                    ALU.add,
                )

            psA_cm.__exit__(None, None, None)

            ctx2 = ExitStack()
            with ctx2:
                psS = ctx2.enter_context(tc.tile_pool(name="psS", bufs=2, space="PSUM"))
                psO = ctx2.enter_context(tc.tile_pool(name="psO", bufs=1, space="PSUM"))

                # ---- phase B: attention, one head at a time ----
                for h in range(H_LOC):
                    pr, po = h // 2, (h % 2) * DIM_HEAD
                    ps_o = psO.tile([DIM_HEAD + 1, N], F32, tag="po")
                    for qh in range(2):
                        qs, qe = qh * HALF, (qh + 1) * HALF
                        for kb in range(NB):
                            qlo = kb * P
                            if qlo >= qe:
                                break
                            s_ps = psS.tile([P, HALF], F32, tag="ps_s")
                            for qt in range(qs // QT, qe // QT):
                                rs, re = qt * QT, (qt + 1) * QT
                                if re <= qlo:
                                    continue
                                nc.tensor.matmul(
                                    s_ps[:, rs - qs:re - qs],
                                    KTt[pr][po:po + DIM_HEAD, qlo:qlo + P],
                                    QTt[pr][po:po + DIM_HEAD, rs:re],
                                    start=True, stop=True,
                                    tile_position=(po, 0),
                                )
                            vstart = max(qlo, qs)
                            ex = expp.tile([P, HALF], BF16, tag="ex")
                            nc.scalar.activation(
                                ex[:, vstart - qs:], s_ps[:, vstart - qs:], AF.Exp
                            )
                            if qlo >= qs:
                                # causal staircase on the diagonal block
                                nc.gpsimd.affine_select(
                                    out=ex[:, qlo - qs:qlo - qs + P],
                                    in_=ex[:, qlo - qs:qlo - qs + P],
                                    compare_op=ALU.is_ge,
                                    fill=0.0,
                                    base=0,
                                    channel_multiplier=-1,
                                    pattern=[[1, P]],
                                )
                            for qt in range(qs // QT, qe // QT):
                                rs, re = qt * QT, (qt + 1) * QT
                                if re <= qlo:
                                    continue
                                cs = max(qlo, rs)
                                nc.tensor.matmul(
                                    ps_o[:, cs:re],
                                    Vt[:, kb, h, :],
                                    ex[:, cs - qs:re - qs],
                                    start=(kb == 0),
                                    stop=(kb == min(NB - 1, re // P - 1)),
                                )
                    # normalize: recip = exp(-log(denom)); out = out * recip
                    lnrow = smalls.tile([1, N], F32, tag="lnrow")
                    nc.scalar.activation(lnrow[:], ps_o[DIM_HEAD:DIM_HEAD + 1, :], AF.Ln)
                    nc.scalar.activation(lnrow[:], lnrow[:], AF.Exp, scale=-1.0)
                    recip_dram = dramp.tile([1, N], F32, tag="recip_dram")
                    nc.sync.dma_start(recip_dram[:], lnrow[:])
                    recip_bc = rbcp.tile([DIM_HEAD, N], F32, tag="recip_bc")
                    nc.sync.dma_start(
                        recip_bc[:], recip_dram[:].to_broadcast((DIM_HEAD, N))
                    )
                    nc.vector.tensor_tensor(
                        outT[pr][po:po + DIM_HEAD, :],
                        ps_o[:DIM_HEAD, :],
                        recip_bc[:],
                        ALU.mult,
                    )

            # ---- phase C: out projection ----
            psP = ctx.enter_context(tc.tile_pool(name="psP", bufs=4, space="PSUM"))
            for qb in range(NB):
                for nt in range(2):
                    ps = psP.tile([P, 512], F32, tag="pp")
                    for pb in range(2):
                        nc.tensor.matmul(
                            ps[:],
                            outT[pb][:, qb * P:(qb + 1) * P],
                            wo_bf[:, pb, nt * 512:(nt + 1) * 512],
                            start=(pb == 0), stop=(pb == 1),
                        )
                    so = stage.tile([P, 512], F32, tag="so")
                    if (qb * 2 + nt) % 2 == 0:
                        nc.scalar.copy(so[:], ps[:])
                    else:
                        nc.vector.tensor_copy(so[:], ps[:])
                    nc.sync.dma_start(
                        out_d[qb * P:(qb + 1) * P, nt * 512:(nt + 1) * 512], so[:]
                    )

    nc.compile()
    return nc


def make_in_maps(x, ln_w, ln_b, w_qkv, w_out):
    x = np.asarray(x, np.float32)
    ln_w = np.asarray(ln_w, np.float32)
    ln_b = np.asarray(ln_b, np.float32)
    w_qkv = np.asarray(w_qkv, np.float32)
    w_out = np.asarray(w_out, np.float32)

    in_maps = []
    for c in range(N_CORES):
        b, g = c // 4, c % 4
        cols = np.arange(4 * g * DIM_HEAD, (4 * g + H_LOC) * DIM_HEAD)
        wq_s = w_qkv[:, cols]
        wk_s = w_qkv[:, INNER + cols]
        wv_s = w_qkv[:, 2 * INNER + cols]
        wq = np.ascontiguousarray(ln_w[:, None] * wq_s * SCALE)
        wk = np.ascontiguousarray(ln_w[:, None] * wk_s)
        wv = np.ascontiguousarray(ln_w[:, None] * wv_s)
        bq = (ln_b @ wq_s) * SCALE
        bk = ln_b @ wk_s
        bv = ln_b @ wv_s
        in_maps.append({
            "x": np.ascontiguousarray(x[b]),
            "wq": wq, "wk": wk, "wv": wv,
            "wo": np.ascontiguousarray(w_out[cols, :]),
            "bq": np.ascontiguousarray(bq.reshape(2, P).T),
            "bk": np.ascontiguousarray(bk.reshape(2, P).T),
            "bv": bv.reshape(1, H_LOC * DIM_HEAD),
        })
    return in_maps


_NC_CACHE = []


def kernel(x, ln_w, ln_b, w_qkv, w_out):
    in_maps = make_in_maps(x, ln_w, ln_b, w_qkv, w_out)
    if not _NC_CACHE:
        _NC_CACHE.append(build_nc())
    nc = _NC_CACHE[0]
    res = run_bass_kernel_spmd(nc, in_maps, list(range(N_CORES))).results
    out = np.zeros((B, N, DIM), np.float32)
    for c in range(N_CORES):
        out[c // 4] += res[c]["out"]
    return out


# revision 12
# speedup vs baseline: 1.1996x; 1.1996x over previous
"""Causal attention (LN -> QKV -> 16-head causal attn -> out-proj) on 8 TRN2 cores.

Sharding: core c = (batch b=c//4, head-group g=c%4). Each core runs its batch's
LayerNorm + a 4-head slice of QKV / attention / out-projection. The out-proj
partials (column-split over the inner dim) are summed on the host per batch.

Device layout notes (per core):
  xnT  4x [128, 8, 512] bf16  normalized input, transposed (DIM on partitions),
                              split by seq-quarter so QKV pipelines into LN
  QT/KT pair tiles [128, 2048] bf16  (two heads stacked: head-dim on partitions)
  V    [128, 16, 4, 65] bf16  natural keys-on-partitions layout; 65th col = 1.0
                              so PV's lhsT also accumulates softmax denominators
  S^T  per (head-pair, q-half, key-block): psum [128 keys, 1024 q]; the two
       heads of a pair run concurrently via tile_position row-packing (d=64).
       exp on ScalarE straight from PSUM; causal staircase = skip fully masked
       regions + one DVE affine_select on the diagonal 128x128 block.
  outT psum [65, 1024]: rows 0..63 = unnormalized head output, row 64 = softmax
       denominators. Denominators land in a [4, 2048] SBUF tile; per-pair
       normalization (recip = exp(-ln d), DRAM-broadcast, DVE multiply) runs
       overlapped with the next pair's attention.
"""

import numpy as np

import concourse.bass as bass
import concourse.mybir as mybir
import concourse.tile as tile
from concourse import bacc
from concourse.bass_utils import run_bass_kernel_spmd
from concourse.masks import make_identity

B, N, DIM, HEADS, DIM_HEAD = 2, 2048, 1024, 16, 64
INNER = HEADS * DIM_HEAD
H_LOC = 4                      # heads per core
N_CORES = 8
P = 128
NB = N // P                    # 16 seq blocks
KB = DIM // P                  # 8 dim blocks
QT = 512                       # psum-bank-sized q tile
HALF = 1024                    # q span per S^T psum tile
SCALE = DIM_HEAD ** -0.5
LN_EPS = 1e-5

F32 = mybir.dt.float32
BF16 = mybir.dt.bfloat16
AF = mybir.ActivationFunctionType
ALU = mybir.AluOpType


def build_nc():
    from contextlib import ExitStack

    nc = bacc.Bacc(None, target_bir_lowering=False, debug=False)

    x_d = nc.dram_tensor("x", [N, DIM], F32, kind="ExternalInput")
    wq_d = nc.dram_tensor("wq", [DIM, H_LOC * DIM_HEAD], F32, kind="ExternalInput")
    wk_d = nc.dram_tensor("wk", [DIM, H_LOC * DIM_HEAD], F32, kind="ExternalInput")
    wv_d = nc.dram_tensor("wv", [DIM, H_LOC * DIM_HEAD], F32, kind="ExternalInput")
    wo_d = nc.dram_tensor("wo", [H_LOC * DIM_HEAD, DIM], F32, kind="ExternalInput")
    bq_d = nc.dram_tensor("bq", [P, 2], F32, kind="ExternalInput")
    bk_d = nc.dram_tensor("bk", [P, 2], F32, kind="ExternalInput")
    bv_d = nc.dram_tensor("bv", [1, H_LOC * DIM_HEAD], F32, kind="ExternalInput")
    out_d = nc.dram_tensor("out", [N, DIM], F32, kind="ExternalOutput")

    with tile.TileContext(nc) as tc:
        ctx = ExitStack()
        with ctx:
            const = ctx.enter_context(tc.tile_pool(name="const", bufs=1))
            persist = ctx.enter_context(tc.tile_pool(name="persist", bufs=1))
            wstage = ctx.enter_context(tc.tile_pool(name="wstage", bufs=2))
            xpool = ctx.enter_context(tc.tile_pool(name="xpool", bufs=3))
            xnpool = ctx.enter_context(tc.tile_pool(name="xnpool", bufs=3))
            stat = ctx.enter_context(tc.tile_pool(name="stat", bufs=4))
            expp = ctx.enter_context(tc.tile_pool(name="expp", bufs=2))
            smalls = ctx.enter_context(tc.tile_pool(name="smalls", bufs=2))
            rbcp = ctx.enter_context(tc.tile_pool(name="rbcp", bufs=2))
            dramp = ctx.enter_context(tc.tile_pool(name="dramp", bufs=2, space="DRAM"))
            stage = ctx.enter_context(tc.tile_pool(name="stage", bufs=3))

            # ---- constants ----
            ident = const.tile([P, P], BF16, tag="ident")
            make_identity(nc, ident)
            # keep-mask for the causal diagonal block: tri[k, q] = (k <= q)
            tri = const.tile([P, P], BF16, tag="tri")
            nc.gpsimd.memset(tri[:], 0.0)
            nc.gpsimd.affine_select(
                out=tri[:], in_=tri[:], compare_op=ALU.is_gt, fill=1.0,
                base=0, channel_multiplier=1, pattern=[[-1, P]],
            )
            eps_t = const.tile([P, 1], F32, tag="eps")
            nc.vector.memset(eps_t, LN_EPS)
            bq_sb = const.tile([P, 2], F32, tag="bq")
            nc.sync.dma_start(bq_sb[:], bq_d[:])
            bk_sb = const.tile([P, 2], F32, tag="bk")
            nc.sync.dma_start(bk_sb[:], bk_d[:])
            bv_sb = const.tile([P, H_LOC, DIM_HEAD], F32, tag="bv")
            nc.sync.dma_start(
                bv_sb[:],
                bv_d[:].rearrange("o (h d) -> o h d", h=H_LOC)
                .to_broadcast((P, H_LOC, DIM_HEAD)),
            )

            xnT = [persist.tile([P, KB, 4 * P], BF16, tag=f"xnT{q}", name=f"xnT{q}")
                   for q in range(4)]
            QTt = [persist.tile([P, N], BF16, tag=f"qt{p_}", name=f"qt{p_}")
                   for p_ in range(2)]
            KTt = [persist.tile([P, N], BF16, tag=f"kt{p_}", name=f"kt{p_}")
                   for p_ in range(2)]
            Vt = persist.tile([P, NB, H_LOC, DIM_HEAD + 1], BF16, tag="v")
            nc.gpsimd.memset(Vt[:], 1.0)  # 65th column stays 1.0 -> denominators
            outT = [persist.tile([P, N], BF16, tag=f"outT{p_}", name=f"outT{p_}")
                    for p_ in range(2)]
            den = [persist.tile([1, N], F32, tag=f"den{h}", name=f"den{h}")
                   for h in range(H_LOC)]

            # ---- phase A: LN -> transpose -> QKV -> V ----
            psA_cm = tc.tile_pool(name="psA", bufs=4, space="PSUM")
            psA = psA_cm.__enter__()

            for sb in range(NB):
                x_t = xpool.tile([P, DIM], F32, tag="x")
                nc.sync.dma_start(x_t[:], x_d[sb * P:(sb + 1) * P, :])

                stats = stat.tile([P, 2, 6], F32, tag="bnst")
                x3 = x_t[:].rearrange("p (a f) -> p a f", a=2)
                for a in range(2):
                    nc.vector.bn_stats(stats[:, a, :], x3[:, a, :])
                mv = stat.tile([P, 2], F32, tag="mv")
                nc.vector.bn_aggr(mv[:], stats[:])
                rstd = stat.tile([P, 1], F32, tag="rstd")
                nc.scalar.activation(rstd[:], mv[:, 1:2], AF.Sqrt, bias=eps_t[:])
                nc.vector.reciprocal(rstd[:], rstd[:])
                negmean = stat.tile([P, 1], F32, tag="negmean")
                nc.vector.tensor_scalar_mul(negmean[:], mv[:, 0:1], -1.0)

                xn_bf = xnpool.tile([P, DIM], BF16, tag="xn")
                nc.vector.tensor_scalar(
                    xn_bf[:], x_t[:], negmean[:], rstd[:], ALU.add, ALU.mult
                )

                # transpose this seq block: 8 dim-blocks via PE, 2 psum tiles
                for half in range(2):
                    ps = psA.tile([P, 512], F32, tag="ps")
                    for j in range(4):
                        kb = half * 4 + j
                        nc.tensor.matmul(
                            ps[:, j * P:(j + 1) * P],
                            xn_bf[:, kb * P:(kb + 1) * P],
                            ident[:],
                            start=True, stop=True,
                        )
                    dst = xnT[sb // 4][:, half * 4:(half + 1) * 4,
                                       (sb % 4) * P:(sb % 4 + 1) * P]
                    nc.scalar.copy(dst, ps[:].rearrange("p (a f) -> p a f", a=4))

            # weights (loaded after x in program order -> lower DMA priority)
            def load_w_bf(dram, shape3, tag):
                st = wstage.tile(shape3, F32, tag="wst", name=f"wst_{tag}")
                nc.sync.dma_start(st[:], dram[:].rearrange("(kb p) m -> p kb m", p=P))
                bf = persist.tile(shape3, BF16, tag=tag, name=f"bf_{tag}")
                nc.gpsimd.tensor_copy(bf[:], st[:])
                return bf

            wq_bf = load_w_bf(wq_d, [P, KB, H_LOC * DIM_HEAD], "wq")
            wk_bf = load_w_bf(wk_d, [P, KB, H_LOC * DIM_HEAD], "wk")
            wv_bf = load_w_bf(wv_d, [P, KB, H_LOC * DIM_HEAD], "wv")
            wo_bf = load_w_bf(wo_d, [P, 2, DIM], "wo")

            # Q^T / K^T per head-pair
            for (wt, bias_sb, dstt) in ((wq_bf, bq_sb, QTt), (wk_bf, bk_sb, KTt)):
                for pr in range(2):
                    for st in range(4):
                        ps = psA.tile([P, 512], F32, tag="ps")
                        for kb in range(KB):
                            nc.tensor.matmul(
                                ps[:],
                                wt[:, kb, pr * P:(pr + 1) * P],
                                xnT[st][:, kb, :],
                                start=(kb == 0), stop=(kb == KB - 1),
                            )
                        nc.scalar.activation(
                            dstt[pr][:, st * 512:(st + 1) * 512], ps[:],
                            AF.Identity, bias=bias_sb[:, pr:pr + 1],
                        )

            # V natural layout
            for sb in range(NB):
                ps = psA.tile([P, 512], F32, tag="ps")
                psv = ps[:, :H_LOC * DIM_HEAD]
                for kb in range(KB):
                    nc.tensor.matmul(
                        psv,
                        xnT[sb // 4][:, kb, (sb % 4) * P:(sb % 4 + 1) * P],
                        wv_bf[:, kb, :],
                        start=(kb == 0), stop=(kb == KB - 1),
                    )
                nc.vector.tensor_tensor(
                    Vt[:, sb, :, :DIM_HEAD],
                    psv.rearrange("p (h d) -> p h d", h=H_LOC),
                    bv_sb[:],
                    ALU.add,
                )

            psA_cm.__exit__(None, None, None)

            # ---- phase B: attention, head pairs via tile_position packing ----
            ctx2 = ExitStack()
            with ctx2:
                psS = ctx2.enter_context(tc.tile_pool(name="psS", bufs=1, space="PSUM"))
                psO = ctx2.enter_context(tc.tile_pool(name="psO", bufs=1, space="PSUM"))

                for pr in range(2):
                    for qh in range(2):
                        qs, qe = qh * HALF, (qh + 1) * HALF
                        ps_o = [psO.tile([DIM_HEAD + 1, HALF], F32,
                                         tag=f"po{hh}", name=f"po{hh}_{pr}_{qh}")
                                for hh in range(2)]
                        for kb in range(NB):
                            qlo = kb * P
                            if qlo >= qe:
                                break
                            s_ps = [psS.tile([P, HALF], F32, tag=f"ps_s{hh}",
                                             name=f"ps_s{hh}_{pr}_{qh}_{kb}")
                                    for hh in range(2)]
                            for qt in range(qs // QT, qe // QT):
                                rs, re = qt * QT, (qt + 1) * QT
                                if re <= qlo:
                                    continue
                                for hh in range(2):
                                    po = hh * DIM_HEAD
                                    nc.tensor.matmul(
                                        s_ps[hh][:, rs - qs:re - qs],
                                        KTt[pr][po:po + DIM_HEAD, qlo:qlo + P],
                                        QTt[pr][po:po + DIM_HEAD, rs:re],
                                        start=True, stop=True,
                                        tile_position=(po, 0),
                                    )
                            vstart = max(qlo, qs)
                            exs = []
                            for hh in range(2):
                                ex = expp.tile([P, HALF], BF16, tag=f"ex{hh}",
                                               name=f"ex{hh}_{pr}_{qh}_{kb}")
                                nc.scalar.activation(
                                    ex[:, vstart - qs:], s_ps[hh][:, vstart - qs:],
                                    AF.Exp,
                                )
                                if qlo >= qs:
                                    nc.vector.tensor_tensor(
                                        ex[:, qlo - qs:qlo - qs + P],
                                        ex[:, qlo - qs:qlo - qs + P],
                                        tri[:],
                                        ALU.mult,
                                    )
                                exs.append(ex)
                            for qt in range(qs // QT, qe // QT):
                                rs, re = qt * QT, (qt + 1) * QT
                                if re <= qlo:
                                    continue
                                cs = max(qlo, rs)
                                for hh in range(2):
                                    nc.tensor.matmul(
                                        ps_o[hh][:, cs - qs:re - qs],
                                        Vt[:, kb, 2 * pr + hh, :],
                                        exs[hh][:, cs - qs:re - qs],
                                        start=(kb == 0),
                                        stop=(kb == min(NB - 1, re // P - 1)),
                                    )
                        # evacuate unnormalized output + denominators (DVE)
                        for hh in range(2):
                            h = 2 * pr + hh
                            nc.vector.tensor_copy(
                                outT[pr][hh * DIM_HEAD:(hh + 1) * DIM_HEAD, qs:qe],
                                ps_o[hh][:DIM_HEAD, :],
                            )
                            nc.vector.tensor_copy(
                                den[h][0:1, qs:qe],
                                ps_o[hh][DIM_HEAD:DIM_HEAD + 1, :],
                            )
                    # normalize this pair (overlaps next pair's attention):
                    # recip = exp(-ln(denom)), broadcast via DRAM, multiply.
                    recip_bc = rbcp.tile([P, N], F32, tag="rbc", name=f"rbc{pr}")
                    for hh in range(2):
                        h = 2 * pr + hh
                        nc.scalar.activation(den[h][:], den[h][:], AF.Ln)
                        nc.scalar.activation(den[h][:], den[h][:], AF.Exp, scale=-1.0)
                        recip_dram = dramp.tile([1, N], F32, tag="rd",
                                                name=f"rd{pr}_{hh}")
                        nc.sync.dma_start(recip_dram[:], den[h][:])
                        nc.sync.dma_start(
                            recip_bc[hh * DIM_HEAD:(hh + 1) * DIM_HEAD, :],
                            recip_dram[:].to_broadcast((DIM_HEAD, N)),
                        )
                    nc.vector.tensor_tensor(
                        outT[pr][:], outT[pr][:], recip_bc[:], ALU.mult
                    )

            # ---- phase C: out projection ----
            psP = ctx.enter_context(tc.tile_pool(name="psP", bufs=3, space="PSUM"))
            for qb in range(NB):
                ps = psP.tile([P, 2, 512], F32, tag="pp")
                for nt in range(2):
                    for pb in range(2):
                        nc.tensor.matmul(
                            ps[:, nt, :],
                            outT[pb][:, qb * P:(qb + 1) * P],
                            wo_bf[:, pb, nt * 512:(nt + 1) * 512],
                            start=(pb == 0), stop=(pb == 1),
                        )
                so = stage.tile([P, DIM], F32, tag="so")
                if qb % 2 == 0:
                    nc.scalar.copy(so[:], ps[:].rearrange("p a f -> p (a f)"))
                else:
                    nc.vector.tensor_copy(so[:], ps[:].rearrange("p a f -> p (a f)"))
                nc.sync.dma_start(out_d[qb * P:(qb + 1) * P, :], so[:])

    nc.compile()
    return nc


def make_in_maps(x, ln_w, ln_b, w_qkv, w_out):
    x = np.asarray(x, np.float32)
    ln_w = np.asarray(ln_w, np.float32)
    ln_b = np.asarray(ln_b, np.float32)
    w_qkv = np.asarray(w_qkv, np.float32)
    w_out = np.asarray(w_out, np.float32)

    in_maps = []
    for c in range(N_CORES):
        b, g = c // 4, c % 4
        cols = np.arange(4 * g * DIM_HEAD, (4 * g + H_LOC) * DIM_HEAD)
        wq_s = w_qkv[:, cols]
        wk_s = w_qkv[:, INNER + cols]
        wv_s = w_qkv[:, 2 * INNER + cols]
        wq = np.ascontiguousarray(ln_w[:, None] * wq_s * SCALE)
        wk = np.ascontiguousarray(ln_w[:, None] * wk_s)
        wv = np.ascontiguousarray(ln_w[:, None] * wv_s)
        bq = (ln_b @ wq_s) * SCALE
        bk = ln_b @ wk_s
        bv = ln_b @ wv_s
        in_maps.append({
            "x": np.ascontiguousarray(x[b]),
            "wq": wq, "wk": wk, "wv": wv,
            "wo": np.ascontiguousarray(w_out[cols, :]),
            "bq": np.ascontiguousarray(bq.reshape(2, P).T),
            "bk": np.ascontiguousarray(bk.reshape(2, P).T),
            "bv": bv.reshape(1, H_LOC * DIM_HEAD),
        })
    return in_maps


_NC_CACHE = []


def kernel(x, ln_w, ln_b, w_qkv, w_out):
    in_maps = make_in_maps(x, ln_w, ln_b, w_qkv, w_out)
    if not _NC_CACHE:
        _NC_CACHE.append(build_nc())
    nc = _NC_CACHE[0]
    res = run_bass_kernel_spmd(nc, in_maps, list(range(N_CORES))).results
    out = np.zeros((B, N, DIM), np.float32)
    for c in range(N_CORES):
        out[c // 4] += res[c]["out"]
    return out


# revision 14
# speedup vs baseline: 1.4171x; 1.1813x over previous
"""Causal attention (LN -> QKV -> 16-head causal attn -> out-proj) on 8 TRN2 cores.

Sharding: core c = (batch b=c//4, head-group g=c%4). Each core runs its batch's
LayerNorm + a 4-head slice of QKV / attention / out-projection. The out-proj
partials (column-split over the inner dim) are summed on the host per batch.

Device layout notes (per core):
  xnT  4x [128, 8, 512] bf16  normalized input, transposed (DIM on partitions),
                              split by seq-quarter so QKV pipelines into LN
  QT/KT pair tiles [128, 2048] bf16  (two heads stacked: head-dim on partitions)
  V    [128, 16, 4, 65] bf16  natural keys-on-partitions layout; 65th col = 1.0
                              so PV's lhsT also accumulates softmax denominators
  S^T  per (head-pair, q-half, key-block): psum [128 keys, 1024 q]; the two
       heads of a pair run concurrently via tile_position row-packing (d=64).
       exp on ScalarE straight from PSUM; causal staircase = skip fully masked
       regions + one DVE affine_select on the diagonal 128x128 block.
  outT psum [65, 1024]: rows 0..63 = unnormalized head output, row 64 = softmax
       denominators. Denominators land in a [4, 2048] SBUF tile; per-pair
       normalization (recip = exp(-ln d), DRAM-broadcast, DVE multiply) runs
       overlapped with the next pair's attention.
"""

import numpy as np

import concourse.bass as bass
import concourse.mybir as mybir
import concourse.tile as tile
from concourse import bacc
from concourse.bass_utils import run_bass_kernel_spmd
from concourse.masks import make_identity

B, N, DIM, HEADS, DIM_HEAD = 2, 2048, 1024, 16, 64
INNER = HEADS * DIM_HEAD
H_LOC = 4                      # heads per core
N_CORES = 8
P = 128
NB = N // P                    # 16 seq blocks
KB = DIM // P                  # 8 dim blocks
QT = 512                       # psum-bank-sized q tile
HALF = 1024                    # q span per S^T psum tile
SCALE = DIM_HEAD ** -0.5
LN_EPS = 1e-5

F32 = mybir.dt.float32
BF16 = mybir.dt.bfloat16
AF = mybir.ActivationFunctionType
ALU = mybir.AluOpType


def build_nc():
    from contextlib import ExitStack

    nc = bacc.Bacc(None, target_bir_lowering=False, debug=False)

    x_d = nc.dram_tensor("x", [N, DIM], F32, kind="ExternalInput")
    wq_d = nc.dram_tensor("wq", [DIM, H_LOC * DIM_HEAD], F32, kind="ExternalInput")
    wk_d = nc.dram_tensor("wk", [DIM, H_LOC * DIM_HEAD], F32, kind="ExternalInput")
    wv_d = nc.dram_tensor("wv", [DIM, H_LOC * DIM_HEAD], F32, kind="ExternalInput")
    wo_d = nc.dram_tensor("wo", [H_LOC * DIM_HEAD, DIM], F32, kind="ExternalInput")
    bq_d = nc.dram_tensor("bq", [P, 2], F32, kind="ExternalInput")
    bk_d = nc.dram_tensor("bk", [P, 2], F32, kind="ExternalInput")
    bv_d = nc.dram_tensor("bv", [1, H_LOC * DIM_HEAD], F32, kind="ExternalInput")
    out_d = nc.dram_tensor("out", [N, DIM], F32, kind="ExternalOutput")

    with tile.TileContext(nc) as tc:
        ctx = ExitStack()
        with ctx:
            const = ctx.enter_context(tc.tile_pool(name="const", bufs=1))
            persist = ctx.enter_context(tc.tile_pool(name="persist", bufs=1))
            wstage = ctx.enter_context(tc.tile_pool(name="wstage", bufs=2))
            xpool = ctx.enter_context(tc.tile_pool(name="xpool", bufs=3))
            xnpool = ctx.enter_context(tc.tile_pool(name="xnpool", bufs=3))
            stat = ctx.enter_context(tc.tile_pool(name="stat", bufs=4))
            expp = ctx.enter_context(tc.tile_pool(name="expp", bufs=2))
            smalls = ctx.enter_context(tc.tile_pool(name="smalls", bufs=2))
            rbcp = ctx.enter_context(tc.tile_pool(name="rbcp", bufs=2))
            dramp = ctx.enter_context(tc.tile_pool(name="dramp", bufs=2, space="DRAM"))
            stage = ctx.enter_context(tc.tile_pool(name="stage", bufs=3))

            # ---- constants ----
            ident = const.tile([P, P], BF16, tag="ident")
            make_identity(nc, ident)
            # keep-mask for the causal diagonal block: tri[k, q] = (k <= q)
            tri = const.tile([P, P], BF16, tag="tri")
            nc.gpsimd.memset(tri[:], 0.0)
            nc.gpsimd.affine_select(
                out=tri[:], in_=tri[:], compare_op=ALU.is_gt, fill=1.0,
                base=0, channel_multiplier=1, pattern=[[-1, P]],
            )
            eps_t = const.tile([P, 1], F32, tag="eps")
            nc.vector.memset(eps_t, LN_EPS)
            bq_sb = const.tile([P, 2], F32, tag="bq")
            nc.sync.dma_start(bq_sb[:], bq_d[:])
            bk_sb = const.tile([P, 2], F32, tag="bk")
            nc.sync.dma_start(bk_sb[:], bk_d[:])
            bv_sb = const.tile([P, H_LOC, DIM_HEAD], F32, tag="bv")
            nc.sync.dma_start(
                bv_sb[:],
                bv_d[:].rearrange("o (h d) -> o h d", h=H_LOC)
                .to_broadcast((P, H_LOC, DIM_HEAD)),
            )

            xnT = [persist.tile([P, KB, 4 * P], BF16, tag=f"xnT{q}", name=f"xnT{q}")
                   for q in range(4)]
            QTt = [persist.tile([P, N], BF16, tag=f"qt{p_}", name=f"qt{p_}")
                   for p_ in range(2)]
            KTt = [persist.tile([P, N], BF16, tag=f"kt{p_}", name=f"kt{p_}")
                   for p_ in range(2)]
            Vt = persist.tile([P, NB, H_LOC, DIM_HEAD + 1], BF16, tag="v")
            nc.gpsimd.memset(Vt[:], 1.0)  # 65th column stays 1.0 -> denominators
            outT = [persist.tile([P, N], BF16, tag=f"outT{p_}", name=f"outT{p_}")
                    for p_ in range(2)]
            den = [persist.tile([1, N], F32, tag=f"den{h}", name=f"den{h}")
                   for h in range(H_LOC)]

            # ---- phase A: LN -> transpose -> QKV -> V (interleaved) ----
            psA_cm = tc.tile_pool(name="psA", bufs=4, space="PSUM")
            psA = psA_cm.__enter__()

            def load_w_bf(dram, shape3, tag, eng):
                st = wstage.tile(shape3, F32, tag="wst", name=f"wst_{tag}")
                nc.sync.dma_start(st[:], dram[:].rearrange("(kb p) m -> p kb m", p=P))
                bf = persist.tile(shape3, BF16, tag=tag, name=f"bf_{tag}")
                if eng == "act":
                    nc.scalar.copy(bf[:], st[:])
                elif eng == "dve":
                    nc.vector.tensor_copy(bf[:], st[:])
                else:
                    nc.gpsimd.tensor_copy(bf[:], st[:])
                return bf

            wq_bf = load_w_bf(wq_d, [P, KB, H_LOC * DIM_HEAD], "wq", "act")
            wk_bf = load_w_bf(wk_d, [P, KB, H_LOC * DIM_HEAD], "wk", "act")
            wv_bf = load_w_bf(wv_d, [P, KB, H_LOC * DIM_HEAD], "wv", "dve")
            wo_bf = load_w_bf(wo_d, [P, 2, DIM], "wo", "gps")

            def emit_qkv_st(st):
                for (wt, bias_sb, dstt) in ((wq_bf, bq_sb, QTt), (wk_bf, bk_sb, KTt)):
                    for pr in range(2):
                        ps = psA.tile([P, 512], F32, tag="ps")
                        for kb in range(KB):
                            nc.tensor.matmul(
                                ps[:],
                                wt[:, kb, pr * P:(pr + 1) * P],
                                xnT[st][:, kb, :],
                                start=(kb == 0), stop=(kb == KB - 1),
                            )
                        nc.scalar.activation(
                            dstt[pr][:, st * 512:(st + 1) * 512], ps[:],
                            AF.Identity, bias=bias_sb[:, pr:pr + 1],
                        )

            for sb in range(NB):
                x_t = xpool.tile([P, DIM], F32, tag="x")
                nc.sync.dma_start(x_t[:], x_d[sb * P:(sb + 1) * P, :])

                stats = stat.tile([P, 2, 6], F32, tag="bnst")
                x3 = x_t[:].rearrange("p (a f) -> p a f", a=2)
                for a in range(2):
                    nc.vector.bn_stats(stats[:, a, :], x3[:, a, :])
                mv = stat.tile([P, 2], F32, tag="mv")
                nc.vector.bn_aggr(mv[:], stats[:])
                rstd = stat.tile([P, 1], F32, tag="rstd")
                nc.scalar.activation(rstd[:], mv[:, 1:2], AF.Sqrt, bias=eps_t[:])
                nc.vector.reciprocal(rstd[:], rstd[:])
                # nmrs = -mean * rstd  -> xn = x*rstd + nmrs on ScalarE
                nmrs = stat.tile([P, 1], F32, tag="nmrs")
                nc.vector.tensor_scalar(
                    nmrs[:], mv[:, 0:1], rstd[:], -1.0, ALU.mult, ALU.mult
                )
                xn_bf = xnpool.tile([P, DIM], BF16, tag="xn")
                nc.scalar.activation(
                    xn_bf[:], x_t[:], AF.Identity, bias=nmrs[:], scale=rstd[:]
                )

                # transpose this seq block: 8 dim-blocks via PE, 2 psum tiles
                for half in range(2):
                    ps = psA.tile([P, 512], F32, tag="ps")
                    for j in range(4):
                        kb = half * 4 + j
                        nc.tensor.matmul(
                            ps[:, j * P:(j + 1) * P],
                            xn_bf[:, kb * P:(kb + 1) * P],
                            ident[:],
                            start=True, stop=True,
                        )
                    dst = xnT[sb // 4][:, half * 4:(half + 1) * 4,
                                       (sb % 4) * P:(sb % 4 + 1) * P]
                    src = ps[:].rearrange("p (a f) -> p a f", a=4)
                    if half == 0:
                        nc.scalar.copy(dst, src)
                    else:
                        nc.vector.tensor_copy(dst, src)

                # V for this seq block
                ps = psA.tile([P, 512], F32, tag="ps")
                psv = ps[:, :H_LOC * DIM_HEAD]
                for kb in range(KB):
                    nc.tensor.matmul(
                        psv,
                        xnT[sb // 4][:, kb, (sb % 4) * P:(sb % 4 + 1) * P],
                        wv_bf[:, kb, :],
                        start=(kb == 0), stop=(kb == KB - 1),
                    )
                nc.vector.tensor_tensor(
                    Vt[:, sb, :, :DIM_HEAD],
                    psv.rearrange("p (h d) -> p h d", h=H_LOC),
                    bv_sb[:],
                    ALU.add,
                )

                if sb % 4 == 3:
                    emit_qkv_st(sb // 4)

            psA_cm.__exit__(None, None, None)

            # ---- phase B: attention, head pairs via tile_position packing ----
            ctx2 = ExitStack()
            with ctx2:
                psS = ctx2.enter_context(tc.tile_pool(name="psS", bufs=1, space="PSUM"))
                psO = ctx2.enter_context(tc.tile_pool(name="psO", bufs=1, space="PSUM"))

                for pr in range(2):
                    for qh in range(2):
                        qs, qe = qh * HALF, (qh + 1) * HALF
                        ps_o = [psO.tile([DIM_HEAD + 1, HALF], F32,
                                         tag=f"po{hh}", name=f"po{hh}_{pr}_{qh}")
                                for hh in range(2)]
                        for kb in range(NB):
                            qlo = kb * P
                            if qlo >= qe:
                                break
                            s_ps = [psS.tile([P, HALF], F32, tag=f"ps_s{hh}",
                                             name=f"ps_s{hh}_{pr}_{qh}_{kb}")
                                    for hh in range(2)]
                            for qt in range(qs // QT, qe // QT):
                                rs, re = qt * QT, (qt + 1) * QT
                                if re <= qlo:
                                    continue
                                for hh in range(2):
                                    po = hh * DIM_HEAD
                                    nc.tensor.matmul(
                                        s_ps[hh][:, rs - qs:re - qs],
                                        KTt[pr][po:po + DIM_HEAD, qlo:qlo + P],
                                        QTt[pr][po:po + DIM_HEAD, rs:re],
                                        start=True, stop=True,
                                        tile_position=(po, 0),
                                    )
                            vstart = max(qlo, qs)
                            exs = []
                            for hh in range(2):
                                ex = expp.tile([P, HALF], BF16, tag=f"ex{hh}",
                                               name=f"ex{hh}_{pr}_{qh}_{kb}")
                                nc.scalar.activation(
                                    ex[:, vstart - qs:], s_ps[hh][:, vstart - qs:],
                                    AF.Exp,
                                )
                                if qlo >= qs:
                                    nc.vector.tensor_tensor(
                                        ex[:, qlo - qs:qlo - qs + P],
                                        ex[:, qlo - qs:qlo - qs + P],
                                        tri[:],
                                        ALU.mult,
                                    )
                                exs.append(ex)
                            for qt in range(qs // QT, qe // QT):
                                rs, re = qt * QT, (qt + 1) * QT
                                if re <= qlo:
                                    continue
                                cs = max(qlo, rs)
                                for hh in range(2):
                                    nc.tensor.matmul(
                                        ps_o[hh][:, cs - qs:re - qs],
                                        Vt[:, kb, 2 * pr + hh, :],
                                        exs[hh][:, cs - qs:re - qs],
                                        start=(kb == 0),
                                        stop=(kb == min(NB - 1, re // P - 1)),
                                    )
                        # evacuate unnormalized output + denominators (DVE)
                        for hh in range(2):
                            h = 2 * pr + hh
                            nc.vector.tensor_copy(
                                outT[pr][hh * DIM_HEAD:(hh + 1) * DIM_HEAD, qs:qe],
                                ps_o[hh][:DIM_HEAD, :],
                            )
                            nc.vector.tensor_copy(
                                den[h][0:1, qs:qe],
                                ps_o[hh][DIM_HEAD:DIM_HEAD + 1, :],
                            )
                    # normalize this pair (overlaps next pair's attention):
                    # reciprocal on DVE in [128,16] layout via DRAM shuffles;
                    # ScalarE never touched (no ACT table swaps).
                    recip_bc = rbcp.tile([P, N], F32, tag="rbc", name=f"rbc{pr}")
                    for hh in range(2):
                        h = 2 * pr + hh
                        da = dramp.tile([1, N], F32, tag="da", name=f"da{pr}_{hh}")
                        nc.sync.dma_start(da[:], den[h][:])
                        denc = stat.tile([P, N // P], F32, tag="denc",
                                         name=f"denc{pr}_{hh}")
                        nc.sync.dma_start(
                            denc[:],
                            da[0, :].rearrange("(p o) -> p o", o=N // P),
                        )
                        nc.vector.reciprocal(denc[:], denc[:])
                        db = dramp.tile([1, N], F32, tag="db", name=f"db{pr}_{hh}")
                        nc.sync.dma_start(
                            db[0, :].rearrange("(p o) -> p o", o=N // P),
                            denc[:],
                        )
                        nc.sync.dma_start(
                            recip_bc[hh * DIM_HEAD:(hh + 1) * DIM_HEAD, :],
                            db[:].to_broadcast((DIM_HEAD, N)),
                        )
                    nc.vector.tensor_tensor(
                        outT[pr][:], outT[pr][:], recip_bc[:], ALU.mult
                    )

            # ---- phase C: out projection ----
            psP = ctx.enter_context(tc.tile_pool(name="psP", bufs=3, space="PSUM"))
            for qb in range(NB):
                ps = psP.tile([P, 2, 512], F32, tag="pp")
                for nt in range(2):
                    for pb in range(2):
                        nc.tensor.matmul(
                            ps[:, nt, :],
                            outT[pb][:, qb * P:(qb + 1) * P],
                            wo_bf[:, pb, nt * 512:(nt + 1) * 512],
                            start=(pb == 0), stop=(pb == 1),
                        )
                so = stage.tile([P, DIM], F32, tag="so")
                if qb % 2 == 0:
                    nc.scalar.copy(so[:], ps[:].rearrange("p a f -> p (a f)"))
                else:
                    nc.vector.tensor_copy(so[:], ps[:].rearrange("p a f -> p (a f)"))
                nc.sync.dma_start(out_d[qb * P:(qb + 1) * P, :], so[:])

    nc.compile()
    return nc


def make_in_maps(x, ln_w, ln_b, w_qkv, w_out):
    x = np.asarray(x, np.float32)
    ln_w = np.asarray(ln_w, np.float32)
    ln_b = np.asarray(ln_b, np.float32)
    w_qkv = np.asarray(w_qkv, np.float32)
    w_out = np.asarray(w_out, np.float32)

    in_maps = []
    for c in range(N_CORES):
        b, g = c // 4, c % 4
        cols = np.arange(4 * g * DIM_HEAD, (4 * g + H_LOC) * DIM_HEAD)
        wq_s = w_qkv[:, cols]
        wk_s = w_qkv[:, INNER + cols]
        wv_s = w_qkv[:, 2 * INNER + cols]
        wq = np.ascontiguousarray(ln_w[:, None] * wq_s * SCALE)
        wk = np.ascontiguousarray(ln_w[:, None] * wk_s)
        wv = np.ascontiguousarray(ln_w[:, None] * wv_s)
        bq = (ln_b @ wq_s) * SCALE
        bk = ln_b @ wk_s
        bv = ln_b @ wv_s
        in_maps.append({
            "x": np.ascontiguousarray(x[b]),
            "wq": wq, "wk": wk, "wv": wv,
            "wo": np.ascontiguousarray(w_out[cols, :]),
            "bq": np.ascontiguousarray(bq.reshape(2, P).T),
            "bk": np.ascontiguousarray(bk.reshape(2, P).T),
            "bv": bv.reshape(1, H_LOC * DIM_HEAD),
        })
    return in_maps


_NC_CACHE = []


def kernel(x, ln_w, ln_b, w_qkv, w_out):
    in_maps = make_in_maps(x, ln_w, ln_b, w_qkv, w_out)
    if not _NC_CACHE:
        _NC_CACHE.append(build_nc())
    nc = _NC_CACHE[0]
    res = run_bass_kernel_spmd(nc, in_maps, list(range(N_CORES))).results
    out = np.zeros((B, N, DIM), np.float32)
    for c in range(N_CORES):
        out[c // 4] += res[c]["out"]
    return out


# revision 17
# speedup vs baseline: 1.5045x; 1.0617x over previous
"""Causal attention (LN -> QKV -> 16-head causal attn -> out-proj) on 8 TRN2 cores.

Sharding: core c = (batch b=c//4, head-group g=c%4). Each core runs its batch's
LayerNorm + a 4-head slice of QKV / attention / out-projection. The out-proj
partials (column-split over the inner dim) are summed on the host per batch.

Device layout notes (per core):
  xnT  4x [128, 8, 512] bf16  normalized input, transposed (DIM on partitions),
                              split by seq-quarter so QKV pipelines into LN
  QT/KT pair tiles [128, 2048] bf16  (two heads stacked: head-dim on partitions)
  V    [128, 16, 4, 65] bf16  natural keys-on-partitions layout; 65th col = 1.0
                              so PV's lhsT also accumulates softmax denominators
  S^T  per (head-pair, q-half, key-block): psum [128 keys, 1024 q]; the two
       heads of a pair run concurrently via tile_position row-packing (d=64).
       exp on ScalarE straight from PSUM; causal staircase = skip fully masked
       regions + one DVE affine_select on the diagonal 128x128 block.
  outT psum [65, 1024]: rows 0..63 = unnormalized head output, row 64 = softmax
       denominators. Denominators land in a [4, 2048] SBUF tile; per-pair
       normalization (recip = exp(-ln d), DRAM-broadcast, DVE multiply) runs
       overlapped with the next pair's attention.
"""

import numpy as np

import concourse.bass as bass
import concourse.mybir as mybir
import concourse.tile as tile
from concourse import bacc
from concourse.bass_utils import run_bass_kernel_spmd
from concourse.masks import make_identity

B, N, DIM, HEADS, DIM_HEAD = 2, 2048, 1024, 16, 64
INNER = HEADS * DIM_HEAD
H_LOC = 4                      # heads per core
N_CORES = 8
P = 128
NB = N // P                    # 16 seq blocks
KB = DIM // P                  # 8 dim blocks
QT = 512                       # psum-bank-sized q tile
HALF = 1024                    # q span per S^T psum tile
SCALE = DIM_HEAD ** -0.5
LN_EPS = 1e-5

F32 = mybir.dt.float32
BF16 = mybir.dt.bfloat16
AF = mybir.ActivationFunctionType
ALU = mybir.AluOpType


def build_nc():
    from contextlib import ExitStack

    nc = bacc.Bacc(None, target_bir_lowering=False, debug=False)

    x_d = nc.dram_tensor("x", [N, DIM], F32, kind="ExternalInput")
    wq_d = nc.dram_tensor("wq", [DIM, H_LOC * DIM_HEAD], F32, kind="ExternalInput")
    wk_d = nc.dram_tensor("wk", [DIM, H_LOC * DIM_HEAD], F32, kind="ExternalInput")
    wv_d = nc.dram_tensor("wv", [DIM, H_LOC * DIM_HEAD], F32, kind="ExternalInput")
    wo_d = nc.dram_tensor("wo", [H_LOC * DIM_HEAD, DIM], F32, kind="ExternalInput")
    bq_d = nc.dram_tensor("bq", [P, 2], F32, kind="ExternalInput")
    bk_d = nc.dram_tensor("bk", [P, 2], F32, kind="ExternalInput")
    bv_d = nc.dram_tensor("bv", [1, H_LOC * DIM_HEAD], F32, kind="ExternalInput")
    out_d = nc.dram_tensor("out", [N, DIM], F32, kind="ExternalOutput")

    with tile.TileContext(nc) as tc:
        ctx = ExitStack()
        with ctx:
            const = ctx.enter_context(tc.tile_pool(name="const", bufs=1))
            persist = ctx.enter_context(tc.tile_pool(name="persist", bufs=1))
            wstage = ctx.enter_context(tc.tile_pool(name="wstage", bufs=2))
            xpool = ctx.enter_context(tc.tile_pool(name="xpool", bufs=5))
            xnpool = ctx.enter_context(tc.tile_pool(name="xnpool", bufs=4))
            stat = ctx.enter_context(tc.tile_pool(name="stat", bufs=8))
            expp = ctx.enter_context(tc.tile_pool(name="expp", bufs=3))
            smalls = ctx.enter_context(tc.tile_pool(name="smalls", bufs=2))
            rbcp = ctx.enter_context(tc.tile_pool(name="rbcp", bufs=2))
            dramp = ctx.enter_context(tc.tile_pool(name="dramp", bufs=2, space="DRAM"))
            stage = ctx.enter_context(tc.tile_pool(name="stage", bufs=3))

            # ---- constants ----
            ident = const.tile([P, P], BF16, tag="ident")
            make_identity(nc, ident)
            # keep-mask for the causal diagonal block: tri[k, q] = (k <= q)
            tri = const.tile([P, P], BF16, tag="tri")
            nc.gpsimd.memset(tri[:], 0.0)
            nc.gpsimd.affine_select(
                out=tri[:], in_=tri[:], compare_op=ALU.is_gt, fill=1.0,
                base=0, channel_multiplier=1, pattern=[[-1, P]],
            )
            eps_t = const.tile([P, 1], F32, tag="eps")
            nc.vector.memset(eps_t, LN_EPS)
            bq_sb = const.tile([P, 2], F32, tag="bq")
            nc.sync.dma_start(bq_sb[:], bq_d[:])
            bk_sb = const.tile([P, 2], F32, tag="bk")
            nc.sync.dma_start(bk_sb[:], bk_d[:])
            bv_sb = const.tile([P, H_LOC, DIM_HEAD], F32, tag="bv")
            nc.sync.dma_start(
                bv_sb[:],
                bv_d[:].rearrange("o (h d) -> o h d", h=H_LOC)
                .to_broadcast((P, H_LOC, DIM_HEAD)),
            )

            xnT = [persist.tile([P, KB, 4 * P], BF16, tag=f"xnT{q}", name=f"xnT{q}")
                   for q in range(4)]
            QTt = [persist.tile([P, N], BF16, tag=f"qt{p_}", name=f"qt{p_}")
                   for p_ in range(2)]
            KTt = [persist.tile([P, N], BF16, tag=f"kt{p_}", name=f"kt{p_}")
                   for p_ in range(2)]
            Vt = persist.tile([P, NB, H_LOC, DIM_HEAD + 1], BF16, tag="v")
            nc.gpsimd.memset(Vt[:], 1.0)  # 65th column stays 1.0 -> denominators
            outT = [[persist.tile([P, HALF], BF16, tag=f"outT{p_}_{q_}",
                                  name=f"outT{p_}_{q_}") for q_ in range(2)]
                    for p_ in range(2)]

            # ---- phase A: LN -> transpose -> QKV -> V (interleaved) ----
            psA_cm = tc.tile_pool(name="psA", bufs=4, space="PSUM")
            psA = psA_cm.__enter__()

            def load_w_bf(dram, shape3, tag, eng):
                st = wstage.tile(shape3, F32, tag="wst", name=f"wst_{tag}")
                nc.sync.dma_start(st[:], dram[:].rearrange("(kb p) m -> p kb m", p=P))
                bf = persist.tile(shape3, BF16, tag=tag, name=f"bf_{tag}")
                if eng == "act":
                    nc.scalar.copy(bf[:], st[:])
                elif eng == "dve":
                    nc.vector.tensor_copy(bf[:], st[:])
                else:
                    nc.gpsimd.tensor_copy(bf[:], st[:])
                return bf

            wq_bf = load_w_bf(wq_d, [P, KB, H_LOC * DIM_HEAD], "wq", "act")
            wk_bf = load_w_bf(wk_d, [P, KB, H_LOC * DIM_HEAD], "wk", "act")
            wv_bf = load_w_bf(wv_d, [P, KB, H_LOC * DIM_HEAD], "wv", "dve")
            wo_bf = load_w_bf(wo_d, [P, 2, DIM], "wo", "gps")

            def emit_qkv_st(st):
                for (wt, bias_sb, dstt) in ((wq_bf, bq_sb, QTt), (wk_bf, bk_sb, KTt)):
                    for pr in range(2):
                        ps = psA.tile([P, 512], F32, tag="ps")
                        for kb in range(KB):
                            nc.tensor.matmul(
                                ps[:],
                                wt[:, kb, pr * P:(pr + 1) * P],
                                xnT[st][:, kb, :],
                                start=(kb == 0), stop=(kb == KB - 1),
                            )
                        nc.vector.tensor_scalar_add(
                            dstt[pr][:, st * 512:(st + 1) * 512], ps[:],
                            bias_sb[:, pr:pr + 1],
                        )

            for sb in range(NB):
                x_t = xpool.tile([P, DIM], F32, tag="x")
                nc.sync.dma_start(x_t[:], x_d[sb * P:(sb + 1) * P, :])

                stats = stat.tile([P, 2, 6], F32, tag="bnst")
                x3 = x_t[:].rearrange("p (a f) -> p a f", a=2)
                for a in range(2):
                    nc.vector.bn_stats(stats[:, a, :], x3[:, a, :])
                mv = stat.tile([P, 2], F32, tag="mv")
                nc.vector.bn_aggr(mv[:], stats[:])
                rstd = stat.tile([P, 1], F32, tag="rstd")
                nc.scalar.activation(rstd[:], mv[:, 1:2], AF.Sqrt, bias=eps_t[:])
                nc.vector.reciprocal(rstd[:], rstd[:])
                # nmrs = -mean * rstd  -> xn = x*rstd + nmrs on ScalarE
                nmrs = stat.tile([P, 1], F32, tag="nmrs")
                nc.vector.tensor_scalar(
                    nmrs[:], mv[:, 0:1], rstd[:], -1.0, ALU.mult, ALU.mult
                )
                xn_bf = xnpool.tile([P, DIM], BF16, tag="xn")
                nc.scalar.activation(
                    xn_bf[:], x_t[:], AF.Identity, bias=nmrs[:], scale=rstd[:]
                )

                # transpose this seq block: 8 dim-blocks via PE, 2 psum tiles
                for half in range(2):
                    ps = psA.tile([P, 512], F32, tag="ps")
                    for j in range(4):
                        kb = half * 4 + j
                        nc.tensor.matmul(
                            ps[:, j * P:(j + 1) * P],
                            xn_bf[:, kb * P:(kb + 1) * P],
                            ident[:],
                            start=True, stop=True,
                        )
                    dst = xnT[sb // 4][:, half * 4:(half + 1) * 4,
                                       (sb % 4) * P:(sb % 4 + 1) * P]
                    src = ps[:].rearrange("p (a f) -> p a f", a=4)
                    if half == 0:
                        nc.scalar.copy(dst, src)
                    else:
                        nc.vector.tensor_copy(dst, src)

                # V for this seq block
                ps = psA.tile([P, 512], F32, tag="ps")
                psv = ps[:, :H_LOC * DIM_HEAD]
                for kb in range(KB):
                    nc.tensor.matmul(
                        psv,
                        xnT[sb // 4][:, kb, (sb % 4) * P:(sb % 4 + 1) * P],
                        wv_bf[:, kb, :],
                        start=(kb == 0), stop=(kb == KB - 1),
                    )
                nc.vector.tensor_tensor(
                    Vt[:, sb, :, :DIM_HEAD],
                    psv.rearrange("p (h d) -> p h d", h=H_LOC),
                    bv_sb[:],
                    ALU.add,
                )

                if sb % 4 == 3:
                    emit_qkv_st(sb // 4)

            psA_cm.__exit__(None, None, None)

            # ---- phase B: attention, head pairs via tile_position packing ----
            ctx2 = ExitStack()
            with ctx2:
                psS = ctx2.enter_context(tc.tile_pool(name="psS", bufs=1, space="PSUM"))
                psO = ctx2.enter_context(tc.tile_pool(name="psO", bufs=1, space="PSUM"))

                for pr in range(2):
                    for qh in range(2):
                        qs, qe = qh * HALF, (qh + 1) * HALF
                        ps_o = [psO.tile([DIM_HEAD + 1, HALF], F32,
                                         tag=f"po{hh}", name=f"po{hh}_{pr}_{qh}")
                                for hh in range(2)]
                        for kb in range(NB):
                            qlo = kb * P
                            if qlo >= qe:
                                break
                            s_ps = [psS.tile([P, HALF], F32, tag=f"ps_s{hh}",
                                             name=f"ps_s{hh}_{pr}_{qh}_{kb}")
                                    for hh in range(2)]
                            for qt in range(qs // QT, qe // QT):
                                rs, re = qt * QT, (qt + 1) * QT
                                if re <= qlo:
                                    continue
                                for hh in range(2):
                                    po = hh * DIM_HEAD
                                    nc.tensor.matmul(
                                        s_ps[hh][:, rs - qs:re - qs],
                                        KTt[pr][po:po + DIM_HEAD, qlo:qlo + P],
                                        QTt[pr][po:po + DIM_HEAD, rs:re],
                                        start=True, stop=True,
                                        tile_position=(po, 0),
                                    )
                            vstart = max(qlo, qs)
                            exs = []
                            for hh in range(2):
                                ex = expp.tile([P, HALF], BF16, tag=f"ex{hh}",
                                               name=f"ex{hh}_{pr}_{qh}_{kb}")
                                nc.scalar.activation(
                                    ex[:, vstart - qs:], s_ps[hh][:, vstart - qs:],
                                    AF.Exp,
                                )
                                if qlo >= qs:
                                    nc.vector.tensor_tensor(
                                        ex[:, qlo - qs:qlo - qs + P],
                                        ex[:, qlo - qs:qlo - qs + P],
                                        tri[:],
                                        ALU.mult,
                                    )
                                exs.append(ex)
                            for qt in range(qs // QT, qe // QT):
                                rs, re = qt * QT, (qt + 1) * QT
                                if re <= qlo:
                                    continue
                                cs = max(qlo, rs)
                                for hh in range(2):
                                    nc.tensor.matmul(
                                        ps_o[hh][:, cs - qs:re - qs],
                                        Vt[:, kb, 2 * pr + hh, :],
                                        exs[hh][:, cs - qs:re - qs],
                                        start=(kb == 0),
                                        stop=(kb == min(NB - 1, re // P - 1)),
                                    )
                        # evacuate unnormalized output (DVE) + denom row (DMA)
                        # and normalize this (pair, q-half) immediately:
                        # reciprocal on DVE in [128,8] layout via DRAM shuffles.
                        recip_bc = rbcp.tile([P, HALF], F32, tag="rbc",
                                             name=f"rbc{pr}_{qh}")
                        for hh in range(2):
                            nc.vector.tensor_copy(
                                outT[pr][qh][hh * DIM_HEAD:(hh + 1) * DIM_HEAD, :],
                                ps_o[hh][:DIM_HEAD, :],
                            )
                            dr = stat.tile([1, HALF], F32, tag="denrow",
                                           name=f"dr{pr}_{qh}_{hh}")
                            nc.vector.tensor_copy(
                                dr[:], ps_o[hh][DIM_HEAD:DIM_HEAD + 1, :]
                            )
                            da = dramp.tile([1, HALF], F32, tag="da",
                                            name=f"da{pr}_{qh}_{hh}")
                            nc.sync.dma_start(da[:], dr[:])
                            denc = stat.tile([P, HALF // P], F32, tag="denc",
                                             name=f"denc{pr}_{qh}_{hh}")
                            nc.sync.dma_start(
                                denc[:],
                                da[0, :].rearrange("(p o) -> p o", o=HALF // P),
                            )
                            nc.vector.reciprocal(denc[:], denc[:])
                            db = dramp.tile([1, HALF], F32, tag="db",
                                            name=f"db{pr}_{qh}_{hh}")
                            nc.sync.dma_start(
                                db[0, :].rearrange("(p o) -> p o", o=HALF // P),
                                denc[:],
                            )
                            nc.sync.dma_start(
                                recip_bc[hh * DIM_HEAD:(hh + 1) * DIM_HEAD, :],
                                db[:].to_broadcast((DIM_HEAD, HALF)),
                            )
                        nc.vector.tensor_tensor(
                            outT[pr][qh][:], outT[pr][qh][:], recip_bc[:], ALU.mult
                        )

            # ---- phase C: out projection ----
            psP = ctx.enter_context(tc.tile_pool(name="psP", bufs=3, space="PSUM"))
            for qb in range(NB):
                ps = psP.tile([P, 2, 512], F32, tag="pp")
                for nt in range(2):
                    for pb in range(2):
                        nc.tensor.matmul(
                            ps[:, nt, :],
                            outT[pb][qb // 8][:, (qb % 8) * P:(qb % 8 + 1) * P],
                            wo_bf[:, pb, nt * 512:(nt + 1) * 512],
                            start=(pb == 0), stop=(pb == 1),
                        )
                so = stage.tile([P, DIM], F32, tag="so")
                if qb % 2 == 0:
                    nc.scalar.copy(so[:], ps[:].rearrange("p a f -> p (a f)"))
                else:
                    nc.vector.tensor_copy(so[:], ps[:].rearrange("p a f -> p (a f)"))
                nc.sync.dma_start(out_d[qb * P:(qb + 1) * P, :], so[:])

    nc.compile()
    return nc


def make_in_maps(x, ln_w, ln_b, w_qkv, w_out):
    x = np.asarray(x, np.float32)
    ln_w = np.asarray(ln_w, np.float32)
    ln_b = np.asarray(ln_b, np.float32)
    w_qkv = np.asarray(w_qkv, np.float32)
    w_out = np.asarray(w_out, np.float32)

    in_maps = []
    for c in range(N_CORES):
        b, g = c // 4, c % 4
        cols = np.arange(4 * g * DIM_HEAD, (4 * g + H_LOC) * DIM_HEAD)
        wq_s = w_qkv[:, cols]
        wk_s = w_qkv[:, INNER + cols]
        wv_s = w_qkv[:, 2 * INNER + cols]
        wq = np.ascontiguousarray(ln_w[:, None] * wq_s * SCALE)
        wk = np.ascontiguousarray(ln_w[:, None] * wk_s)
        wv = np.ascontiguousarray(ln_w[:, None] * wv_s)
        bq = (ln_b @ wq_s) * SCALE
        bk = ln_b @ wk_s
        bv = ln_b @ wv_s
        in_maps.append({
            "x": np.ascontiguousarray(x[b]),
            "wq": wq, "wk": wk, "wv": wv,
            "wo": np.ascontiguousarray(w_out[cols, :]),
            "bq": np.ascontiguousarray(bq.reshape(2, P).T),
            "bk": np.ascontiguousarray(bk.reshape(2, P).T),
            "bv": bv.reshape(1, H_LOC * DIM_HEAD),
        })
    return in_maps


_NC_CACHE = []


def kernel(x, ln_w, ln_b, w_qkv, w_out):
    in_maps = make_in_maps(x, ln_w, ln_b, w_qkv, w_out)
    if not _NC_CACHE:
        _NC_CACHE.append(build_nc())
    nc = _NC_CACHE[0]
    res = run_bass_kernel_spmd(nc, in_maps, list(range(N_CORES))).results
    out = np.zeros((B, N, DIM), np.float32)
    for c in range(N_CORES):
        out[c // 4] += res[c]["out"]
    return out
